# revision 26
# baseline (speedup 1.0000x reference)
"""Trainium2 Bass kernel for CrossBranchAttentionWithSA.

Sharding: 8 cores = 2 batches x 4 query-chunks. Each core processes an
864-query window (576 own queries + halo rows so the 7x7 SpatialAttention
conv sees its neighborhood), with the full kv sequence of its batch.

Device schedule (per core):
 1. V = kv_input @ wv (keys-on-partitions, per-head 65-col layout whose 65th
    column is ones: the AV matmul then also yields the softmax denominator).
 2. Q.T/K.T tile 0, then attention heads 0..11; Q.T/K.T tile t+1 is emitted
    between heads so the (ScalarE-exp-bound) attention phase absorbs the
    projection matmuls in TensorE slack. Q.T/K.T live in per-tile tensors so
    head h only depends on tile h//2.
 3. Per head: S.T = K_h.T^T Q_h.T -> exp (scale folded) -> AV.T accumulation
    over key tiles; denominator division via a DRAM-roundtrip row broadcast.
 4. proj consumes attn.T as stationary operand giving untransposed [query,
    channel] rows; mean/max channel stats via free-dim reduces; conv as 7
    shifted K=14 matmuls over a gutter-padded row layout; sigmoid; per-query
    scale; store.
"""
import os
import numpy as np
import ml_dtypes

import concourse.bass as bass
import concourse.bacc as bacc
import concourse.tile as tile
from concourse import mybir
from concourse.bass_utils import run_bass_kernel_spmd

F32 = mybir.dt.float32
BF16 = mybir.dt.bfloat16
AF = mybir.ActivationFunctionType
AX = mybir.AxisListType
bf16 = ml_dtypes.bfloat16

DIM, HEADS, HGT, WID = 768, 12, 48, 48
HD = DIM // HEADS          # 64
N = HGT * WID              # 2304
SA_K = 7
B = 2
W = 864                    # window queries per core (18 image rows)
OWNQ = 576
ROWS_W = W // WID          # 18
MC = WID + 6               # 54 (gutter-padded row width)
MPW = (ROWS_W + 12) * MC   # padded map span incl top/bottom zero rows + tail
CONV_SPAN = ROWS_W * MC    # 972

W0 = [0, 432, 1008, 1440]                    # window starts per chunk
OWN0 = [576 * c - W0[c] for c in range(4)]   # own-row offset inside window

SCALE = float(HD) ** -0.5


def build_program():
    nc = bacc.Bacc("TRN2", target_bir_lowering=False, debug=False,
                   enable_asserts=False, num_devices=8)

    xq_t = nc.dram_tensor("xq_t", [DIM, W], BF16, kind="ExternalInput").ap()
    xkv_t = nc.dram_tensor("xkv_t", [DIM, N], BF16, kind="ExternalInput").ap()
    wq_t = nc.dram_tensor("wq_t", [DIM, DIM], BF16, kind="ExternalInput").ap()
    wk_t = nc.dram_tensor("wk_t", [DIM, DIM], BF16, kind="ExternalInput").ap()
    wv_t = nc.dram_tensor("wv_t", [DIM, DIM], BF16, kind="ExternalInput").ap()
    wp_t = nc.dram_tensor("wp_t", [DIM, DIM], BF16, kind="ExternalInput").ap()
    qb_d = nc.dram_tensor("qb", [6, 128], F32, kind="ExternalInput").ap()
    kb_d = nc.dram_tensor("kb", [6, 128], F32, kind="ExternalInput").ap()
    vb_d = nc.dram_tensor("vb", [1, DIM], F32, kind="ExternalInput").ap()
    pb_d = nc.dram_tensor("pb", [1, DIM], F32, kind="ExternalInput").ap()
    saw_d = nc.dram_tensor("saw", [14, SA_K], F32, kind="ExternalInput").ap()
    out_d = nc.dram_tensor("out", [W, DIM], F32, kind="ExternalOutput").ap()

    with tile.TileContext(nc) as tc:
        build_tile(tc, xq_t, xkv_t, wq_t, wk_t, wv_t, wp_t,
                   qb_d, kb_d, vb_d, pb_d, saw_d, out_d)
    nc.compile()
    return nc


def build_tile(tc, xq_t, xkv_t, wq_t, wk_t, wv_t, wp_t,
               qb_d, kb_d, vb_d, pb_d, saw_d, out_d):
    nc = tc.nc

    with tc.tile_pool(name="big", bufs=1) as big:
        # ---------- load inputs (V-path tensors first) ----------
        ins_pool = tc.tile_pool(name="ins", bufs=1)
        ins = ins_pool.__enter__()
        wv_sb = ins.tile([128, 6, DIM], BF16, tag="wv")
        nc.sync.dma_start(wv_sb[:], wv_t.rearrange("(t p) m -> p t m", p=128))
        xkv_sb = ins.tile([128, 6, N], BF16, tag="xkv")
        nc.sync.dma_start(xkv_sb[:], xkv_t.rearrange("(t p) m -> p t m", p=128))
        wq_sb = ins.tile([128, 6, DIM], BF16, tag="wq")
        nc.sync.dma_start(wq_sb[:], wq_t.rearrange("(t p) m -> p t m", p=128))
        xq_sb = ins.tile([128, 6, W], BF16, tag="xq")
        nc.sync.dma_start(xq_sb[:], xq_t.rearrange("(t p) m -> p t m", p=128))
        wk_sb = ins.tile([128, 6, DIM], BF16, tag="wk")
        nc.sync.dma_start(wk_sb[:], wk_t.rearrange("(t p) m -> p t m", p=128))
        wp_sb = big.tile([128, 6, DIM], BF16, tag="wp")
        nc.sync.dma_start(wp_sb[:], wp_t.rearrange("(t p) m -> p t m", p=128))

        qb_sb = big.tile([128, 6], F32, tag="qb")
        nc.sync.dma_start(qb_sb[:], qb_d.rearrange("t p -> p t"))
        kb_sb = big.tile([128, 6], F32, tag="kb")
        nc.sync.dma_start(kb_sb[:], kb_d.rearrange("t p -> p t"))
        saw_sb = big.tile([14, SA_K], F32, tag="saw")
        nc.sync.dma_start(saw_sb[:], saw_d)
        vb_bc = big.tile([128, DIM], F32, tag="vbb")
        nc.sync.dma_start(vb_bc[:], bass.AP(
            tensor=vb_d.tensor, offset=0, ap=[[0, 128], [1, DIM]]))
        pb_bc = big.tile([128, DIM], F32, tag="pbb")
        nc.sync.dma_start(pb_bc[:], bass.AP(
            tensor=pb_d.tensor, offset=0, ap=[[0, 128], [1, DIM]]))

        # pre-touch DMA-loaded tiles on DVE+ACT so later instructions inherit
        # the DMA sem ticks instead of each re-waiting
        touch = big.tile([128, 4], F32, tag="touch")
        for ap in (xq_sb[:, 0, 0:2], xkv_sb[:, 0, 0:2], wq_sb[:, 0, 0:2],
                   wk_sb[:, 0, 0:2], wv_sb[:, 0, 0:2], wp_sb[:, 0, 0:2],
                   qb_sb[:, 0:2], kb_sb[:, 0:2], vb_bc[:, 0:2],
                   pb_bc[:, 0:2]):
            nc.vector.tensor_copy(touch[:, 0:2], ap)
            nc.scalar.copy(touch[:, 2:4], ap)
        nc.vector.tensor_copy(touch[0:14, 0:2], saw_sb[:, 0:2])
        nc.scalar.copy(touch[0:14, 2:4], saw_sb[:, 0:2])

        # ---------- projection targets ----------
        qts = [big.tile([128, W], BF16, tag="qt%d" % t, name="qt%d" % t)
               for t in range(6)]
        kts = [big.tile([128, N], BF16, tag="kt%d" % t, name="kt%d" % t)
               for t in range(6)]
        v_sb = big.tile([128, 18, 65 * HEADS], BF16, tag="v")
        nc.vector.memset(
            v_sb[:].rearrange("p t (h x) -> p t h x", x=65)[:, :, :, 64:65], 1.0)

        def emit_qt(t, pool, psz):
            ps = pool.tile([128, psz], F32, tag="pj")
            for i0, iw in ((0, 512), (512, 352)):
                for ct in range(6):
                    nc.tensor.matmul(ps[:, i0:i0 + iw],
                                     wq_sb[:, ct, 128 * t:128 * (t + 1)],
                                     xq_sb[:, ct, i0:i0 + iw],
                                     start=(ct == 0), stop=(ct == 5))
            nc.vector.tensor_scalar_add(qts[t][:], ps[:, 0:W],
                                        qb_sb[:, t:t + 1])

        def emit_kt(t, pool, psz, chunk):
            for c0 in range(0, N, chunk):
                ps = pool.tile([128, psz], F32, tag="pj")
                segs = [(i, min(512, chunk - i)) for i in range(0, chunk, 512)]
                for i0, iw in segs:
                    for ct in range(6):
                        nc.tensor.matmul(ps[:, i0:i0 + iw],
                                         wk_sb[:, ct, 128 * t:128 * (t + 1)],
                                         xkv_sb[:, ct, c0 + i0:c0 + i0 + iw],
                                         start=(ct == 0), stop=(ct == 5))
                nc.vector.tensor_scalar_add(kts[t][:, c0:c0 + chunk],
                                            ps[:, 0:chunk], kb_sb[:, t:t + 1])

        # ---------- V projection, then first Q/K tiles ----------
        with tc.tile_pool(name="pj", bufs=2, space="PSUM") as pj:
            for mt in range(18):
                ps = pj.tile([128, 1152], F32, tag="pj")
                for o0, ow in ((0, 512), (512, 256)):
                    for ct in range(6):
                        nc.tensor.matmul(
                            ps[:, o0:o0 + ow],
                            xkv_sb[:, ct, 128 * mt:128 * (mt + 1)],
                            wv_sb[:, ct, o0:o0 + ow],
                            start=(ct == 0), stop=(ct == 5))
                nc.vector.tensor_add(
                    v_sb[:, mt].rearrange("p (h x) -> p h x", x=65)[:, :, 0:64],
                    ps[:, 0:DIM].rearrange("p (h x) -> p h x", x=64),
                    vb_bc[:].rearrange("p (h x) -> p h x", x=64))
            emit_qt(0, pj, 1152)
            emit_kt(0, pj, 1152, 1152)

        # ---------- attention, with remaining projections interleaved ------
        attn_sb = big.tile([128, 6, W], BF16, tag="attn")
        with (
            tc.tile_pool(name="st", bufs=2, space="PSUM") as stp,
            tc.tile_pool(name="av", bufs=1, space="PSUM") as avp,
            tc.tile_pool(name="pj2", bufs=1, space="PSUM") as pj2,
            tc.tile_pool(name="pt", bufs=3) as ptp,
            tc.tile_pool(name="fin", bufs=1) as finp,
            tc.tile_pool(name="drd", bufs=2, space="DRAM") as drdp,
        ):
            for h in range(HEADS):
                t, bp = h // 2, 64 * (h % 2)
                if h % 2 == 1 and t + 1 < 6:
                    emit_qt(t + 1, pj2, 1024)
                    emit_kt(t + 1, pj2, 1024, 768)
                qt_h = qts[t][bp:bp + HD, :]
                kt_h = kts[t][bp:bp + HD, :]
                av = avp.tile([65, W], F32, tag="av")
                for jt in range(18):
                    st = stp.tile([128, W], F32, tag="st")
                    lhs = kt_h[:, 128 * jt:128 * (jt + 1)]
                    nc.tensor.matmul(st[:, 0:512], lhs, qt_h[:, 0:512],
                                     start=True, stop=True)
                    nc.tensor.matmul(st[:, 512:W], lhs, qt_h[:, 512:W],
                                     start=True, stop=True)
                    pt = ptp.tile([128, W], BF16, tag="pt")
                    nc.scalar.activation(pt[:], st[:], AF.Exp, scale=SCALE)
                    vh = v_sb[:, jt, 65 * h:65 * h + 65]
                    nc.tensor.matmul(av[:, 0:512], vh, pt[:, 0:512],
                                     start=(jt == 0), stop=(jt == 17))
                    nc.tensor.matmul(av[:, 512:W], vh, pt[:, 512:W],
                                     start=(jt == 0), stop=(jt == 17))
                recip = finp.tile([1, W], F32, tag="recip")
                nc.vector.reciprocal(recip[:], av[64:65, :])
                rd = drdp.tile([1, W], F32, tag="rd")
                nc.sync.dma_start(rd[:], recip[:])
                bc = finp.tile([64, W], F32, tag="bc")
                rap = rd[:]
                nc.sync.dma_start(bc[:], bass.AP(
                    tensor=rap.tensor, offset=rap.offset,
                    ap=[[0, HD], [1, W]]))
                nc.vector.tensor_mul(attn_sb[bp:bp + HD, t, :],
                                     av[0:HD, :], bc[:])

        ins_pool.__exit__(None, None, None)

        # ---------- proj + spatial attention ----------
        out_sb = big.tile([128, 7, DIM], F32, tag="out")
        ssum = big.tile([128, 7], F32, tag="ssum")
        smax = big.tile([128, 7], F32, tag="smax")
        nc.vector.memset(ssum[:], 0.0)
        nc.vector.memset(smax[:], 0.0)
        with tc.tile_pool(name="pp", bufs=2, space="PSUM") as ppp:
            for it in range(7):
                iw = 128 if it < 6 else 96
                pp = ppp.tile([128, DIM], F32, tag="pp")
                for o0, ow in ((0, 512), (512, 256)):
                    for ct in range(6):
                        nc.tensor.matmul(
                            pp[:iw, o0:o0 + ow],
                            attn_sb[:, ct, 128 * it:128 * it + iw],
                            wp_sb[:, ct, o0:o0 + ow],
                            start=(ct == 0), stop=(ct == 5))
                nc.vector.tensor_add(out_sb[:iw, it, :], pp[:iw, 0:DIM],
                                     pb_bc[:iw, :])
                nc.vector.reduce_sum(ssum[:iw, it:it + 1], out_sb[:iw, it, :],
                                     axis=AX.X)
                nc.vector.reduce_max(smax[:iw, it:it + 1], out_sb[:iw, it, :],
                                     axis=AX.X)

            # SpatialAttention: stats -> DRAM transpose -> gutter maps -> conv
            with tc.tile_pool(name="dram", bufs=1, space="DRAM") as drp:
                sc_i = drp.tile([2, 896], F32, tag="sci")
                sc_s = drp.tile([1, W], F32, tag="scs")
                mprime = big.tile([2, MPW], F32, tag="mp")
                nc.vector.memset(mprime[:], 0.0)
                for ch, stat in ((0, ssum), (1, smax)):
                    # [128,7] col-major stats -> linear i = 128*t + p
                    nc.sync.dma_start(
                        sc_i[ch].rearrange("(b a) -> a b", b=7), stat[:, 0:7])
                    # i-order rows -> SBUF gutter layout (offset 3 rows+3 col)
                    sl = mprime[ch:ch + 1, 3 * MC + 3:3 * MC + 3 + WID]
                    dst = bass.AP(tensor=sl.tensor, offset=sl.offset,
                                  ap=[list(sl.ap[0]), [MC, ROWS_W], [1, WID]])
                    nc.sync.dma_start(
                        dst,
                        sc_i[ch, 0:W].rearrange("(o r c) -> o r c",
                                                o=1, c=WID))
                # A'[(ci,ky), q] = mprime[ci, ky*MC + q]  (overlapping rows)
                aprime = big.tile([14, CONV_SPAN + 6], F32, tag="ap")
                for ci in range(2):
                    for ky in range(SA_K):
                        nc.sync.dma_start(
                            aprime[ci * SA_K + ky:ci * SA_K + ky + 1, :],
                            mprime[ci:ci + 1,
                                   ky * MC:ky * MC + CONV_SPAN + 6])
                # conv = 7 shifted K=14 matmuls
                cps = ppp.tile([1, CONV_SPAN], F32, tag="cps")
                for s0, sw in ((0, 512), (512, CONV_SPAN - 512)):
                    for kx in range(SA_K):
                        nc.tensor.matmul(cps[:, s0:s0 + sw],
                                         saw_sb[:, kx:kx + 1],
                                         aprime[:, kx + s0:kx + s0 + sw],
                                         start=(kx == 0), stop=(kx == 6))
                sig_row = big.tile([1, CONV_SPAN], F32, tag="sigr")
                nc.scalar.activation(sig_row[:], cps[:], AF.Sigmoid)
                sig_clean = big.tile([1, W], F32, tag="sigc")
                sr = sig_row[:, 0:WID]
                sig_src = bass.AP(tensor=sr.tensor, offset=sr.offset,
                                  ap=[list(sr.ap[0]), [MC, ROWS_W], [1, WID]])
                nc.vector.tensor_copy(
                    sig_clean[:].rearrange("p (r c) -> p r c", c=WID), sig_src)
                nc.sync.dma_start(sc_s[:], sig_clean[:])
                sig_col = big.tile([128, 7], F32, tag="sigcol")
                nc.sync.dma_start(
                    sig_col[:, 0:6],
                    sc_s[0, 0:768].rearrange("(b a) -> a b", b=6))
                nc.sync.dma_start(
                    sig_col[0:96, 6:7],
                    sc_s[0, 768:W].rearrange("(a b) -> a b", b=1))
            for it in range(7):
                iw = 128 if it < 6 else 96
                nc.vector.tensor_scalar_mul(out_sb[:iw, it, :],
                                            out_sb[:iw, it, :],
                                            sig_col[:iw, it:it + 1])
                if it < 6:
                    nc.sync.dma_start(
                        out_d[128 * it:128 * (it + 1)], out_sb[:, it, :])
                else:
                    nc.sync.dma_start(out_d[768:W], out_sb[0:96, 6, :])


_NC = None
LAST_RESULTS = None


def _get_nc():
    global _NC
    if _NC is None:
        _NC = build_program()
    return _NC


def kernel(q_input, kv_input, q_w, q_b, kv_w, kv_b, proj_w, proj_b, sa_w):
    f32 = np.float32
    q_input = np.asarray(q_input, f32)
    kv_input = np.asarray(kv_input, f32)
    wq_t = np.ascontiguousarray(np.asarray(q_w, f32).T).astype(bf16)
    wk_t = np.ascontiguousarray(np.asarray(kv_w, f32)[:DIM].T).astype(bf16)
    wv_t = np.ascontiguousarray(np.asarray(kv_w, f32)[DIM:].T).astype(bf16)
    wp_t = np.ascontiguousarray(np.asarray(proj_w, f32).T).astype(bf16)
    qb = np.asarray(q_b, f32).reshape(6, 128)
    kb = np.asarray(kv_b, f32)[:DIM].reshape(6, 128)
    vb = np.asarray(kv_b, f32)[DIM:].reshape(1, DIM)
    pb = np.asarray(proj_b, f32).reshape(1, DIM)
    sa = np.asarray(sa_w, f32)[0].copy()          # [2, 7, 7]
    sa[0] /= DIM                                  # fold 1/768 mean scale
    saw = np.ascontiguousarray(sa.reshape(14, SA_K))

    shared = dict(wq_t=wq_t, wk_t=wk_t, wv_t=wv_t, wp_t=wp_t,
                  qb=qb, kb=kb, vb=vb, pb=pb, saw=saw)
    in_maps = []
    for b in range(B):
        xkv = np.ascontiguousarray(kv_input[b].T).astype(bf16)
        for c in range(4):
            w0 = W0[c]
            xq = np.ascontiguousarray(q_input[b, w0:w0 + W].T).astype(bf16)
            in_maps.append(dict(xq_t=xq, xkv_t=xkv, **shared))

    res = run_bass_kernel_spmd(_get_nc(), in_maps, core_ids=list(range(8)))
    global LAST_RESULTS
    LAST_RESULTS = res
    out = np.zeros((B, N, DIM), dtype=f32)
    for b in range(B):
        for c in range(4):
            core_out = res.results[4 * b + c]["out"]
            out[b, 576 * c:576 * (c + 1)] = \
                core_out[OWN0[c]:OWN0[c] + OWNQ]
    return out


# revision 28
# speedup vs baseline: 1.0936x; 1.0936x over previous
"""Trainium2 Bass kernel for CrossBranchAttentionWithSA.

Sharding: 8 cores = 2 batches x 4 query-chunks. Each core processes an
864-query window (576 own queries + halo rows so the 7x7 SpatialAttention
conv sees its neighborhood), with the full kv sequence of its batch.

Device schedule (per core):
 1. V = kv_input @ wv (keys-on-partitions, per-head 65-col layout whose 65th
    column is ones: the AV matmul then also yields the softmax denominator).
 2. Q.T/K.T tile 0, then attention heads 0..11; Q.T/K.T tile t+1 is emitted
    between heads so the (ScalarE-exp-bound) attention phase absorbs the
    projection matmuls in TensorE slack. Q.T/K.T live in per-tile tensors so
    head h only depends on tile h//2.
 3. Per head: S.T = K_h.T^T Q_h.T -> exp (scale folded) -> AV.T accumulation
    over key tiles; denominator division via a DRAM-roundtrip row broadcast.
 4. proj consumes attn.T as stationary operand giving untransposed [query,
    channel] rows; mean/max channel stats via free-dim reduces; conv as 7
    shifted K=14 matmuls over a gutter-padded row layout; sigmoid; per-query
    scale; store.
"""
import os
import numpy as np
import ml_dtypes

import concourse.bass as bass
import concourse.bacc as bacc
import concourse.tile as tile
from concourse import mybir
from concourse.bass_utils import run_bass_kernel_spmd

F32 = mybir.dt.float32
BF16 = mybir.dt.bfloat16
AF = mybir.ActivationFunctionType
AX = mybir.AxisListType
bf16 = ml_dtypes.bfloat16

DIM, HEADS, HGT, WID = 768, 12, 48, 48
HD = DIM // HEADS          # 64
N = HGT * WID              # 2304
SA_K = 7
B = 2
W = 864                    # window queries per core (18 image rows)
OWNQ = 576
ROWS_W = W // WID          # 18
MC = WID + 6               # 54 (gutter-padded row width)
MPW = (ROWS_W + 12) * MC   # padded map span incl top/bottom zero rows + tail
CONV_SPAN = ROWS_W * MC    # 972

W0 = [0, 432, 1008, 1440]                    # window starts per chunk
OWN0 = [576 * c - W0[c] for c in range(4)]   # own-row offset inside window

SCALE = float(HD) ** -0.5


def build_program():
    nc = bacc.Bacc("TRN2", target_bir_lowering=False, debug=False,
                   enable_asserts=False, num_devices=8)

    xq_t = nc.dram_tensor("xq_t", [DIM, W], BF16, kind="ExternalInput").ap()
    xkv_t = nc.dram_tensor("xkv_t", [DIM, N], BF16, kind="ExternalInput").ap()
    wq_t = nc.dram_tensor("wq_t", [DIM, DIM], BF16, kind="ExternalInput").ap()
    wk_t = nc.dram_tensor("wk_t", [DIM, DIM], BF16, kind="ExternalInput").ap()
    wv_t = nc.dram_tensor("wv_t", [DIM, DIM], BF16, kind="ExternalInput").ap()
    wp_t = nc.dram_tensor("wp_t", [DIM, DIM], BF16, kind="ExternalInput").ap()
    qb_d = nc.dram_tensor("qb", [6, 128], F32, kind="ExternalInput").ap()
    kb_d = nc.dram_tensor("kb", [6, 128], F32, kind="ExternalInput").ap()
    vb_d = nc.dram_tensor("vb", [1, DIM], F32, kind="ExternalInput").ap()
    pb_d = nc.dram_tensor("pb", [1, DIM], F32, kind="ExternalInput").ap()
    saw_d = nc.dram_tensor("saw", [14, SA_K], F32, kind="ExternalInput").ap()
    out_d = nc.dram_tensor("out", [W, DIM], F32, kind="ExternalOutput").ap()

    with tile.TileContext(nc) as tc:
        build_tile(tc, xq_t, xkv_t, wq_t, wk_t, wv_t, wp_t,
                   qb_d, kb_d, vb_d, pb_d, saw_d, out_d)
    nc.compile()
    return nc


def build_tile(tc, xq_t, xkv_t, wq_t, wk_t, wv_t, wp_t,
               qb_d, kb_d, vb_d, pb_d, saw_d, out_d):
    nc = tc.nc

    with tc.tile_pool(name="big", bufs=1) as big:
        # ---------- load inputs (V-path tensors first) ----------
        ins_pool = tc.tile_pool(name="ins", bufs=1)
        ins = ins_pool.__enter__()
        wv_sb = ins.tile([128, 6, DIM], BF16, tag="wv")
        for c in range(2):
            nc.sync.dma_start(
                wv_sb[:, 3 * c:3 * (c + 1)],
                wv_t.rearrange("(t p) m -> p t m", p=128)[:, 3 * c:3 * (c + 1)])
        xkv_sb = ins.tile([128, 6, N], BF16, tag="xkv")
        for c in range(3):
            nc.sync.dma_start(
                xkv_sb[:, 2 * c:2 * (c + 1)],
                xkv_t.rearrange("(t p) m -> p t m", p=128)[:, 2 * c:2 * (c + 1)])
        wq_sb = ins.tile([128, 6, DIM], BF16, tag="wq")
        nc.sync.dma_start(wq_sb[:], wq_t.rearrange("(t p) m -> p t m", p=128))
        xq_sb = ins.tile([128, 6, W], BF16, tag="xq")
        nc.sync.dma_start(xq_sb[:], xq_t.rearrange("(t p) m -> p t m", p=128))
        wk_sb = ins.tile([128, 6, DIM], BF16, tag="wk")
        nc.sync.dma_start(wk_sb[:], wk_t.rearrange("(t p) m -> p t m", p=128))
        wp_sb = big.tile([128, 6, DIM], BF16, tag="wp")
        nc.sync.dma_start(wp_sb[:], wp_t.rearrange("(t p) m -> p t m", p=128))

        qb_sb = big.tile([128, 6], F32, tag="qb")
        nc.sync.dma_start(qb_sb[:], qb_d.rearrange("t p -> p t"))
        kb_sb = big.tile([128, 6], F32, tag="kb")
        nc.sync.dma_start(kb_sb[:], kb_d.rearrange("t p -> p t"))
        saw_sb = big.tile([14, SA_K], F32, tag="saw")
        nc.sync.dma_start(saw_sb[:], saw_d)
        vb_bc = big.tile([128, DIM], F32, tag="vbb")
        nc.sync.dma_start(vb_bc[:], bass.AP(
            tensor=vb_d.tensor, offset=0, ap=[[0, 128], [1, DIM]]))
        pb_bc = big.tile([128, DIM], F32, tag="pbb")
        nc.sync.dma_start(pb_bc[:], bass.AP(
            tensor=pb_d.tensor, offset=0, ap=[[0, 128], [1, DIM]]))

        # pre-touch DMA-loaded tiles on DVE+ACT so later instructions inherit
        # the DMA sem ticks instead of each re-waiting
        touch = big.tile([128, 4], F32, tag="touch")
        for ap in (xq_sb[:, 0, 0:2], xkv_sb[:, 0, 0:2], wq_sb[:, 0, 0:2],
                   wk_sb[:, 0, 0:2], wv_sb[:, 0, 0:2], wp_sb[:, 0, 0:2],
                   qb_sb[:, 0:2], kb_sb[:, 0:2], vb_bc[:, 0:2],
                   pb_bc[:, 0:2]):
            nc.vector.tensor_copy(touch[:, 0:2], ap)
            nc.scalar.copy(touch[:, 2:4], ap)
        nc.vector.tensor_copy(touch[0:14, 0:2], saw_sb[:, 0:2])
        nc.scalar.copy(touch[0:14, 2:4], saw_sb[:, 0:2])

        # ---------- projection targets ----------
        qts = [big.tile([128, W], BF16, tag="qt%d" % t, name="qt%d" % t)
               for t in range(6)]
        kts = [big.tile([128, N], BF16, tag="kt%d" % t, name="kt%d" % t)
               for t in range(6)]
        v_sb = big.tile([128, 18, 65 * HEADS], BF16, tag="v")
        nc.vector.memset(
            v_sb[:].rearrange("p t (h x) -> p t h x", x=65)[:, :, :, 64:65], 1.0)

        def proj_chunks(t):
            """5 chunks for tile t: 2x Q.T halves + 3x K.T thirds. Each is
            (weights, src_col0, width, dst_ap, bias). Emitted with a
            transiently-borrowed st-pool slot."""
            out = []
            for i0, iw in ((0, 512), (512, 352)):
                out.append((wq_sb, xq_sb, i0, iw, qts[t][:, i0:i0 + iw],
                            qb_sb[:, t:t + 1], t))
            for c0 in range(0, N, DIM):
                out.append((wk_sb, xkv_sb, c0, DIM,
                            kts[t][:, c0:c0 + DIM], kb_sb[:, t:t + 1], t))
            return out

        def emit_chunk(pool, spec):
            w_sb, x_sb, c0, cw, dst, bias, t = spec
            ps = pool.tile([128, 1024], F32, tag="st")
            for i0, iw in [(i, min(512, cw - i)) for i in range(0, cw, 512)]:
                for ct in range(6):
                    nc.tensor.matmul(ps[:, i0:i0 + iw],
                                     w_sb[:, ct, 128 * t:128 * (t + 1)],
                                     x_sb[:, ct, c0 + i0:c0 + i0 + iw],
                                     start=(ct == 0), stop=(ct == 5))
            nc.vector.tensor_scalar_add(dst, ps[:, 0:cw], bias)

        def emit_qt(t, pool):
            for spec in proj_chunks(t)[0:2]:
                emit_chunk(pool, spec)

        def emit_kt(t, pool):
            for spec in proj_chunks(t)[2:5]:
                emit_chunk(pool, spec)

        # ---------- V projection, then first Q/K tiles ----------
        with tc.tile_pool(name="pj", bufs=2, space="PSUM") as pj:
            for mt in range(18):
                ps = pj.tile([128, 1024], F32, tag="st")
                for o0, ow in ((0, 512), (512, 256)):
                    for ct in range(6):
                        nc.tensor.matmul(
                            ps[:, o0:o0 + ow],
                            xkv_sb[:, ct, 128 * mt:128 * (mt + 1)],
                            wv_sb[:, ct, o0:o0 + ow],
                            start=(ct == 0), stop=(ct == 5))
                nc.vector.tensor_add(
                    v_sb[:, mt].rearrange("p (h x) -> p h x", x=65)[:, :, 0:64],
                    ps[:, 0:DIM].rearrange("p (h x) -> p h x", x=64),
                    vb_bc[:].rearrange("p (h x) -> p h x", x=64))
            for spec in proj_chunks(0):
                emit_chunk(pj, spec)

        # ---------- attention, with remaining projections interleaved ------
        attn_sb = big.tile([128, 6, W], BF16, tag="attn")
        with (
            tc.tile_pool(name="st", bufs=2, space="PSUM") as stp,
            tc.tile_pool(name="av", bufs=2, space="PSUM") as avp,
            tc.tile_pool(name="pt", bufs=3) as ptp,
            tc.tile_pool(name="fin", bufs=1) as finp,
            tc.tile_pool(name="drd", bufs=2, space="DRAM") as drdp,
        ):
            for h in range(HEADS):
                t, bp = h // 2, 64 * (h % 2)
                pending = (proj_chunks(t + 1)
                           if (h % 2 == 1 and t + 1 < 6) else [])
                qt_h = qts[t][bp:bp + HD, :]
                kt_h = kts[t][bp:bp + HD, :]
                av = avp.tile([65, W], F32, tag="av")
                for jt in range(18):
                    st = stp.tile([128, W], F32, tag="st")
                    lhs = kt_h[:, 128 * jt:128 * (jt + 1)]
                    nc.tensor.matmul(st[:, 0:512], lhs, qt_h[:, 0:512],
                                     start=True, stop=True)
                    nc.tensor.matmul(st[:, 512:W], lhs, qt_h[:, 512:W],
                                     start=True, stop=True)
                    pt = ptp.tile([128, W], BF16, tag="pt")
                    nc.scalar.activation(pt[:], st[:], AF.Exp, scale=SCALE)
                    vh = v_sb[:, jt, 65 * h:65 * h + 65]
                    nc.tensor.matmul(av[:, 0:512], vh, pt[:, 0:512],
                                     start=(jt == 0), stop=(jt == 17))
                    nc.tensor.matmul(av[:, 512:W], vh, pt[:, 512:W],
                                     start=(jt == 0), stop=(jt == 17))
                    if pending and jt in (2, 5, 8, 11, 14):
                        emit_chunk(stp, pending.pop(0))
                recip = finp.tile([1, W], F32, tag="recip")
                nc.vector.reciprocal(recip[:], av[64:65, :])
                rd = drdp.tile([1, W], F32, tag="rd")
                nc.sync.dma_start(rd[:], recip[:])
                bc = finp.tile([64, W], F32, tag="bc")
                rap = rd[:]
                nc.sync.dma_start(bc[:], bass.AP(
                    tensor=rap.tensor, offset=rap.offset,
                    ap=[[0, HD], [1, W]]))
                nc.vector.tensor_mul(attn_sb[bp:bp + HD, t, :],
                                     av[0:HD, :], bc[:])

        ins_pool.__exit__(None, None, None)

        # ---------- proj + spatial attention ----------
        out_sb = big.tile([128, 7, DIM], F32, tag="out")
        ssum = big.tile([128, 7], F32, tag="ssum")
        smax = big.tile([128, 7], F32, tag="smax")
        nc.vector.memset(ssum[:], 0.0)
        nc.vector.memset(smax[:], 0.0)
        with tc.tile_pool(name="pp", bufs=2, space="PSUM") as ppp:
            for it in range(7):
                iw = 128 if it < 6 else 96
                pp = ppp.tile([128, DIM], F32, tag="pp")
                for o0, ow in ((0, 512), (512, 256)):
                    for ct in range(6):
                        nc.tensor.matmul(
                            pp[:iw, o0:o0 + ow],
                            attn_sb[:, ct, 128 * it:128 * it + iw],
                            wp_sb[:, ct, o0:o0 + ow],
                            start=(ct == 0), stop=(ct == 5))
                nc.vector.tensor_add(out_sb[:iw, it, :], pp[:iw, 0:DIM],
                                     pb_bc[:iw, :])
                nc.vector.reduce_sum(ssum[:iw, it:it + 1], out_sb[:iw, it, :],
                                     axis=AX.X)
                nc.vector.reduce_max(smax[:iw, it:it + 1], out_sb[:iw, it, :],
                                     axis=AX.X)

            # SpatialAttention: stats -> DRAM transpose -> gutter maps -> conv
            with tc.tile_pool(name="dram", bufs=1, space="DRAM") as drp:
                sc_i = drp.tile([2, 896], F32, tag="sci")
                sc_s = drp.tile([1, W], F32, tag="scs")
                mprime = big.tile([2, MPW], F32, tag="mp")
                nc.vector.memset(mprime[:], 0.0)
                for ch, stat in ((0, ssum), (1, smax)):
                    # [128,7] col-major stats -> linear i = 128*t + p
                    nc.sync.dma_start(
                        sc_i[ch].rearrange("(b a) -> a b", b=7), stat[:, 0:7])
                    # i-order rows -> SBUF gutter layout (offset 3 rows+3 col)
                    sl = mprime[ch:ch + 1, 3 * MC + 3:3 * MC + 3 + WID]
                    dst = bass.AP(tensor=sl.tensor, offset=sl.offset,
                                  ap=[list(sl.ap[0]), [MC, ROWS_W], [1, WID]])
                    nc.sync.dma_start(
                        dst,
                        sc_i[ch, 0:W].rearrange("(o r c) -> o r c",
                                                o=1, c=WID))
                # A'[(ci,ky), q] = mprime[ci, ky*MC + q]  (overlapping rows)
                aprime = big.tile([14, CONV_SPAN + 6], F32, tag="ap")
                for ci in range(2):
                    for ky in range(SA_K):
                        nc.sync.dma_start(
                            aprime[ci * SA_K + ky:ci * SA_K + ky + 1, :],
                            mprime[ci:ci + 1,
                                   ky * MC:ky * MC + CONV_SPAN + 6])
                # conv = 7 shifted K=14 matmuls
                cps = ppp.tile([1, CONV_SPAN], F32, tag="cps")
                for s0, sw in ((0, 512), (512, CONV_SPAN - 512)):
                    for kx in range(SA_K):
                        nc.tensor.matmul(cps[:, s0:s0 + sw],
                                         saw_sb[:, kx:kx + 1],
                                         aprime[:, kx + s0:kx + s0 + sw],
                                         start=(kx == 0), stop=(kx == 6))
                sig_row = big.tile([1, CONV_SPAN], F32, tag="sigr")
                nc.scalar.activation(sig_row[:], cps[:], AF.Sigmoid)
                sig_clean = big.tile([1, W], F32, tag="sigc")
                sr = sig_row[:, 0:WID]
                sig_src = bass.AP(tensor=sr.tensor, offset=sr.offset,
                                  ap=[list(sr.ap[0]), [MC, ROWS_W], [1, WID]])
                nc.vector.tensor_copy(
                    sig_clean[:].rearrange("p (r c) -> p r c", c=WID), sig_src)
                nc.sync.dma_start(sc_s[:], sig_clean[:])
                sig_col = big.tile([128, 7], F32, tag="sigcol")
                nc.sync.dma_start(
                    sig_col[:, 0:6],
                    sc_s[0, 0:768].rearrange("(b a) -> a b", b=6))
                nc.sync.dma_start(
                    sig_col[0:96, 6:7],
                    sc_s[0, 768:W].rearrange("(a b) -> a b", b=1))
            for it in range(7):
                iw = 128 if it < 6 else 96
                nc.vector.tensor_scalar_mul(out_sb[:iw, it, :],
                                            out_sb[:iw, it, :],
                                            sig_col[:iw, it:it + 1])
                if it < 6:
                    nc.sync.dma_start(
                        out_d[128 * it:128 * (it + 1)], out_sb[:, it, :])
                else:
                    nc.sync.dma_start(out_d[768:W], out_sb[0:96, 6, :])


_NC = None
LAST_RESULTS = None


def _get_nc():
    global _NC
    if _NC is None:
        _NC = build_program()
    return _NC


def kernel(q_input, kv_input, q_w, q_b, kv_w, kv_b, proj_w, proj_b, sa_w):
    f32 = np.float32
    q_input = np.asarray(q_input, f32)
    kv_input = np.asarray(kv_input, f32)
    wq_t = np.ascontiguousarray(np.asarray(q_w, f32).T).astype(bf16)
    wk_t = np.ascontiguousarray(np.asarray(kv_w, f32)[:DIM].T).astype(bf16)
    wv_t = np.ascontiguousarray(np.asarray(kv_w, f32)[DIM:].T).astype(bf16)
    wp_t = np.ascontiguousarray(np.asarray(proj_w, f32).T).astype(bf16)
    qb = np.asarray(q_b, f32).reshape(6, 128)
    kb = np.asarray(kv_b, f32)[:DIM].reshape(6, 128)
    vb = np.asarray(kv_b, f32)[DIM:].reshape(1, DIM)
    pb = np.asarray(proj_b, f32).reshape(1, DIM)
    sa = np.asarray(sa_w, f32)[0].copy()          # [2, 7, 7]
    sa[0] /= DIM                                  # fold 1/768 mean scale
    saw = np.ascontiguousarray(sa.reshape(14, SA_K))

    shared = dict(wq_t=wq_t, wk_t=wk_t, wv_t=wv_t, wp_t=wp_t,
                  qb=qb, kb=kb, vb=vb, pb=pb, saw=saw)
    in_maps = []
    for b in range(B):
        xkv = np.ascontiguousarray(kv_input[b].T).astype(bf16)
        for c in range(4):
            w0 = W0[c]
            xq = np.ascontiguousarray(q_input[b, w0:w0 + W].T).astype(bf16)
            in_maps.append(dict(xq_t=xq, xkv_t=xkv, **shared))

    res = run_bass_kernel_spmd(_get_nc(), in_maps, core_ids=list(range(8)))
    global LAST_RESULTS
    LAST_RESULTS = res
    out = np.zeros((B, N, DIM), dtype=f32)
    for b in range(B):
        for c in range(4):
            core_out = res.results[4 * b + c]["out"]
            out[b, 576 * c:576 * (c + 1)] = \
                core_out[OWN0[c]:OWN0[c] + OWNQ]
    return out
